# revision 14
# baseline (speedup 1.0000x reference)
"""DSMIL forward pass on 8 Trainium2 NeuronCores (Bass/Tile), bf16 compute.

Sharding: data-parallel over bags, each bag split across a core pair
(core 2b: instances [0:4096) of bag b, core 2b+1: [4096:8192)). ONE tiny
pair-local collective (critical-instance exchange); the final softmax
partial reduction is done on the HOST from per-core (num|den) partials.

v2.2 structure (vs the 154us baseline):
  - h_nat comes from 4 batched XBAR DMA transposes (InstDmaTransposeAnt,
    [128,4096] -> [128,32,128] each, one per d-block of a db-major h^T)
    instead of 128 PE transposes: ~23us of PE work moves to the DMA
    engines. XBARs are serialized against collectives by the framework,
    so they are deliberately emitted where they fire AFTER the warm-up
    AllGather has completed (~50us) - right when the encoder finishes.
  - class scores are computed TRANSPOSED (cls^T[2, n] = W_i^T @ h^T,
    4 512-col matmuls per chunk with a tiny W_i weight load) instead of
    128 natural-layout matmuls each paying a full 128x128 h^T weight
    load (~21us -> ~7.5us of PE). Max/onehot run on the 2-partition
    rows (DVE + gpsimd split), no gpsimd partition_all_reduce.
  - the exchange payload carries m^T + max broadcast [128, 10] so the
    winner-take-all select after the AllGather is 3 tiny DVE ops and
    feeds the q_fn matmuls directly (no PE transposes of m_win).
  - most q-passes (Q = q_fn(h)) are deferred until after the collective
    launch so ~9us of PE work hides the AllGather latency.
  - e^T rows are computed with one 512-col matmul per chunk (q_win as a
    2-col weight load), exp'd on 2 partitions, XBAR'd to natural
    layout; the numerator accumulates e^T @ h_nat per chunk; den is a
    DVE reduction of e^T (exactly the same values the numerator used).
  - the second collective is gone: each core ships (num|den) [C, D+1];
    the host sums the pair and divides.
"""
import numpy as np
import ml_dtypes
from contextlib import ExitStack

import concourse.bacc as bacc
import concourse.tile as tile
import concourse.mybir as mybir

F32 = mybir.dt.float32
BF16 = mybir.dt.bfloat16
AF = mybir.ActivationFunctionType
ALU = mybir.AluOpType
bfdt = ml_dtypes.bfloat16

N_CORES = 8
B_BAGS = 4
N_FULL = 8192
N_LOC = N_FULL // 2

_cache = {}


def _build_kernel(n_cores=N_CORES, N_loc=N_LOC, I=1024, D=512, QD=128,
                  C=2, CHUNK=512, N_WARM0=28, N_WARM=16):
    NB = N_loc // 128          # n-blocks (32)
    NCH = N_loc // CHUNK       # chunks (8)
    BPC = CHUNK // 128         # n-blocks per chunk (4)
    IB = I // 128              # i-blocks (8)
    DB = D // 128              # d-blocks (4)
    assert QD == 128 and C == 2
    inv_sqrt_q = 1.0 / float(np.sqrt(QD))

    nc = bacc.Bacc("TRN2", target_bir_lowering=False, debug=False,
                   num_devices=n_cores)

    xt_d = nc.dram_tensor("xt", [NCH, 128, IB, CHUNK], BF16,
                          kind="ExternalInput")
    w_enc = nc.dram_tensor("w_enc", [128, IB, D], BF16, kind="ExternalInput")
    w_i = nc.dram_tensor("w_i", [128, DB, C], BF16, kind="ExternalInput")
    w_q1 = nc.dram_tensor("w_q1", [128, DB, QD], BF16, kind="ExternalInput")
    w_q2 = nc.dram_tensor("w_q2", [QD, QD], BF16, kind="ExternalInput")
    bias_d = nc.dram_tensor("bias", [128, DB + 2], F32, kind="ExternalInput")
    identb_d = nc.dram_tensor("identb", [128, 128], BF16,
                              kind="ExternalInput")
    identf_d = nc.dram_tensor("identf", [128, 128], F32,
                              kind="ExternalInput")
    out_d = nc.dram_tensor("out", [C, D + 1], F32, kind="ExternalOutput")

    groups = [[i, i + 1] for i in range(0, n_cores, 2)]
    PAY = 2 * DB + C           # payload cols: m^T (DB*C) | max bcast (C)

    with tile.TileContext(nc) as tc, ExitStack() as ctx:
        persist = ctx.enter_context(tc.tile_pool(name="persist", bufs=1))
        dram = ctx.enter_context(tc.tile_pool(name="dram", bufs=1,
                                              space="DRAM"))

        # ---- scratch consts on the (idle) vector queue ----
        scrap = persist.tile([128, 128], BF16)
        nc.vector.memset(scrap[:], 0.0)
        warm_in = dram.tile([1, 2], F32)
        nc.scalar.dma_start(warm_in[:], identf_d[0:1, 0:2])

        # ---- consolidated weight loads (gpsimd queue) ----
        w_enc_sb = persist.tile([128, IB, D], BF16)
        nc.gpsimd.dma_start(w_enc_sb[:, 0:IB // 2, :], w_enc[:, 0:IB // 2, :])
        nc.gpsimd.dma_start(w_enc_sb[:, IB // 2:, :], w_enc[:, IB // 2:, :])
        w_q1_sb = persist.tile([128, DB, QD], BF16)
        nc.gpsimd.dma_start(w_q1_sb[:], w_q1[:])
        w_i_sb = persist.tile([128, DB, C], BF16)
        nc.gpsimd.dma_start(w_i_sb[:], w_i[:])
        w_q2_sb = persist.tile([QD, QD], BF16)
        nc.gpsimd.dma_start(w_q2_sb[:], w_q2[:])

        # warm both collective channels (fires once weights are queued)
        warm_out = dram.tile([2, 2], F32)
        nc.gpsimd.collective_compute(
            "AllGather", ALU.bypass, replica_groups=groups,
            ins=[warm_in[:].opt()], outs=[warm_out[:].opt()])

        # ---- small consts (scalar queue) ----
        identb = persist.tile([128, 128], BF16)
        nc.scalar.dma_start(identb[:], identb_d[:])
        identf = persist.tile([128, 128], F32)
        nc.scalar.dma_start(identf[:], identf_d[:])
        bias_sb = persist.tile([128, DB + 2], F32)
        nc.scalar.dma_start(bias_sb[:], bias_d[:])

        # ---- persistent activations ----
        ht_all = persist.tile([128, DB, NCH, CHUNK], BF16)   # h^T db-major
        h_nat = persist.tile([128, NB, DB, 128], BF16)       # h natural
        qt = persist.tile([128, NCH, CHUNK], BF16)           # Q^T
        clsT = persist.tile([C, NCH, CHUNK], F32)            # scores^T
        ohT = persist.tile([16, NCH, CHUNK], BF16)           # onehot^T (pad)
        eT = persist.tile([16, NCH, CHUNK], BF16)            # e^T (padded)
        oh_nat = persist.tile([128, NB, 16], BF16)
        e_nat = persist.tile([128, NCH, BPC, 16], BF16)
        rmax8 = persist.tile([C, NCH], F32)                  # chunk maxes
        dsum = persist.tile([C, NCH], F32)                   # den partials

        # pad rows of the small-transpose sources are never read back
        # (matmuls slice [:, 0:C]), but zero them so the XBAR never moves
        # uninitialized SBUF.
        nc.vector.memset(ohT[:], 0.0)
        nc.vector.memset(eT[:], 0.0)

        # ================= phase A: encoder + clsT + chunk maxes =======
        with (
            tc.tile_pool(name="xload", bufs=2) as xload,
            tc.tile_pool(name="wp", bufs=1, space="PSUM") as wp,
            tc.tile_pool(name="hp", bufs=2, space="PSUM") as hp,
            tc.tile_pool(name="cp", bufs=2, space="PSUM") as cp,
        ):
            # pre-warm the PE clock gate while the first DMAs land
            pw = wp.tile([128, 128], BF16, name="pw")
            for k in range(N_WARM0):
                nc.tensor.transpose(pw[:], scrap[:], scrap[:])

            for cb in range(NCH):
                xt_c = xload.tile([128, IB, CHUNK], BF16, tag="x", name="x")
                if cb == 0:
                    nc.sync.dma_start(xt_c[:, 0:IB // 2, :],
                                      xt_d[cb][:, 0:IB // 2, :])
                    nc.scalar.dma_start(xt_c[:, IB // 2:, :],
                                        xt_d[cb][:, IB // 2:, :])
                else:
                    nc.sync.dma_start(xt_c[:], xt_d[cb])

                for db in range(DB):
                    ph = hp.tile([128, CHUNK], F32, tag="h", name="h")
                    for ib in range(IB):
                        nc.tensor.matmul(
                            ph[:],
                            w_enc_sb[:, ib, db * 128:(db + 1) * 128],
                            xt_c[:, ib, :],
                            start=(ib == 0), stop=(ib == IB - 1))
                    nc.scalar.activation(ht_all[:, db, cb, :], ph[:],
                                         AF.Relu,
                                         bias=bias_sb[:, db:db + 1])

                # cls^T: [C, 512] psum, W_i as a tiny weight load
                pc = cp.tile([C, CHUNK], F32, tag="c", name="c")
                for db in range(DB):
                    nc.tensor.matmul(pc[:], w_i_sb[:, db, :],
                                     ht_all[:, db, cb, :],
                                     start=(db == 0), stop=(db == DB - 1))
                nc.scalar.copy(clsT[:, cb, :], pc[:])
                nc.vector.reduce_max(rmax8[:, cb:cb + 1], clsT[:, cb, :],
                                     axis=mybir.AxisListType.X)

            # h_nat XBARs: the XBAR ucode is only exact for <=512 input
            # free elements, so one [128,512]->[128,4,128] per (db, cb).
            # Emitted post-loop so they fire after the warm collective
            # (XBARs serialize against in-flight collectives).
            for db in range(DB):
                for cb in range(NCH):
                    eng = nc.sync if (db * NCH + cb) % 2 == 0 else nc.scalar
                    eng.dma_start_transpose(
                        h_nat[:, cb * BPC:(cb + 1) * BPC, db, :],
                        ht_all[:, db, cb, :])

            # global max -> onehot^T (compare split over DVE + gpsimd)
            gmax = persist.tile([C, 1], F32)
            nc.vector.reduce_max(gmax[:], rmax8[:],
                                 axis=mybir.AxisListType.X)
            h8 = NCH // 2
            nc.vector.tensor_scalar(
                ohT[0:C, 0:h8, :].rearrange("p a b -> p (a b)"),
                clsT[:, 0:h8, :].rearrange("p a b -> p (a b)"),
                gmax[:], None, ALU.is_equal)
            nc.gpsimd.tensor_scalar(
                ohT[0:C, h8:, :].rearrange("p a b -> p (a b)"),
                clsT[:, h8:, :].rearrange("p a b -> p (a b)"),
                gmax[:], None, ALU.is_equal)
            for cb in range(NCH):
                eng = nc.sync if cb % 2 == 0 else nc.scalar
                eng.dma_start_transpose(
                    oh_nat[:, cb * BPC:(cb + 1) * BPC, :], ohT[:, cb, :])

        # ====== m extraction + exchange; q-passes hide the AllGather ===
        with (
            tc.tile_pool(name="pmp", bufs=1, space="PSUM") as pmp,
            tc.tile_pool(name="pt", bufs=1, space="PSUM") as pt,
            tc.tile_pool(name="zp", bufs=2, space="PSUM") as zp,
            tc.tile_pool(name="qp", bufs=2, space="PSUM") as qp,
            tc.tile_pool(name="zs", bufs=2) as zs,
        ):
            def q_pass(cb):
                pz = zp.tile([128, CHUNK], F32, tag="z", name="z")
                for db in range(DB):
                    nc.tensor.matmul(pz[:], w_q1_sb[:, db, :],
                                     ht_all[:, db, cb, :],
                                     start=(db == 0), stop=(db == DB - 1))
                zt = zs.tile([128, CHUNK], BF16, tag="zt", name="zt")
                nc.vector.tensor_scalar(zt[:], pz[:],
                                        bias_sb[:, DB:DB + 1], 0.0,
                                        ALU.add, ALU.max)
                pq = qp.tile([128, CHUNK], F32, tag="q", name="q")
                nc.tensor.matmul(pq[:], w_q2_sb[:], zt[:], start=True,
                                 stop=True)
                nc.scalar.activation(qt[:, cb, :], pq[:], AF.Tanh,
                                     bias=bias_sb[:, DB + 1:DB + 2])

            # PE filler while the onehot XBAR lands
            q_pass(0)
            q_pass(1)

            # Tile drops some XBAR-completion edges for multi-reader
            # consumers (observed: the m matmuls waited on only 1 of the 5
            # XBARs); a strict barrier restores them. Everything before it
            # has finished by now anyway.
            tc.strict_bb_all_engine_barrier()

            # m = onehot^T @ h (critical instance features), [C, D] psum
            pm = pmp.tile([C, D], F32, tag="m", name="pm")
            for nb in range(NB):
                nc.tensor.matmul(pm[:], oh_nat[:, nb, 0:C],
                                 h_nat[:, nb, :, :].rearrange(
                                     "p a b -> p (a b)"),
                                 start=(nb == 0), stop=(nb == NB - 1))
            m_sb = persist.tile([C, D], F32)
            nc.scalar.copy(m_sb[:], pm[:])

            # payload = [ m^T (DB x C cols) | max broadcast (C cols) ]
            pay_sb = persist.tile([128, PAY], F32)
            for db in range(DB):
                ptm = pt.tile([128, C], F32, tag="t", name="ptm")
                nc.tensor.transpose(ptm[:],
                                    m_sb[:, db * 128:(db + 1) * 128],
                                    identf[0:C, 0:C])
                nc.scalar.copy(pay_sb[:, db * C:(db + 1) * C], ptm[:])
            pmx = pt.tile([1, C], F32, tag="x", name="pmx")
            nc.tensor.transpose(pmx[:], gmax[:], identf[0:C, 0:C])
            mx_sb = persist.tile([1, C], F32)
            nc.scalar.copy(mx_sb[:], pmx[:])
            maxb = persist.tile([128, C], F32)
            nc.gpsimd.partition_broadcast(maxb[:], mx_sb[:])
            nc.scalar.copy(pay_sb[:, DB * C:], maxb[:])

            pay1 = dram.tile([128, PAY], F32)
            nc.scalar.dma_start(pay1[:], pay_sb[:])
            gath1 = dram.tile([2 * 128, PAY], F32)
            nc.gpsimd.collective_compute(
                "AllGather", ALU.bypass, replica_groups=groups,
                ins=[pay1[:].opt()], outs=[gath1[:].opt()])

            # deferred q-passes cover the collective
            for cb in range(2, NCH):
                q_pass(cb)

            # keep the PE clock gate warm while waiting on the collective
            pwm = pt.tile([128, 128], BF16, tag="w", name="pwm")
            for k in range(N_WARM):
                nc.tensor.transpose(pwm[:], identb[:], identb[:])

        # ================= phase B: winner, q_fn, e, num/den ===========
        with (
            tc.tile_pool(name="pb", bufs=1, space="PSUM") as pb,
            tc.tile_pool(name="ep", bufs=2, space="PSUM") as ep,
            tc.tile_pool(name="pn", bufs=1, space="PSUM") as pn,
        ):
            g2 = persist.tile([128, 2, PAY], F32)
            nc.sync.dma_start(
                g2[:], gath1[:].rearrange("(two p) f -> p two f", p=128))

            # winner-take-all merge (identical result on both cores)
            msk2 = persist.tile([128, 1, C], mybir.dt.uint8)
            nc.vector.tensor_tensor(msk2[:, 0, :], g2[:, 0, DB * C:],
                                    g2[:, 1, DB * C:], ALU.is_ge)
            m_winT = persist.tile([128, DB, C], F32)
            nc.vector.tensor_copy(
                m_winT[:], g2[:, 1, 0:DB * C].rearrange(
                    "p (db c) -> p db c", c=C))
            nc.vector.copy_predicated(
                m_winT[:], msk2[:].broadcast_to([128, DB, C]),
                g2[:, 0, 0:DB * C].rearrange("p (db c) -> p db c", c=C))
            m_winb = persist.tile([128, DB, C], BF16)
            nc.vector.tensor_copy(m_winb[:], m_winT[:])

            # q_win = q_fn(m_win)
            pzm = pb.tile([128, C], F32, tag="pzm", name="pzm")
            for db in range(DB):
                nc.tensor.matmul(pzm[:], w_q1_sb[:, db, :],
                                 m_winb[:, db, :],
                                 start=(db == 0), stop=(db == DB - 1))
            zm = persist.tile([128, C], BF16)
            nc.scalar.activation(zm[:], pzm[:], AF.Relu,
                                 bias=bias_sb[:, DB:DB + 1])
            pqc = pb.tile([128, C], F32, tag="pqc", name="pqc")
            nc.tensor.matmul(pqc[:], w_q2_sb[:], zm[:], start=True,
                             stop=True)
            q_win = persist.tile([128, C], BF16)
            nc.scalar.activation(q_win[:], pqc[:], AF.Tanh,
                                 bias=bias_sb[:, DB + 1:DB + 2])

            # e^T rows -> exp -> XBAR to natural; numerator accumulates
            pnum = pn.tile([C, D], F32, name="pnum")

            def emit_eT(cb):
                pat = ep.tile([C, CHUNK], F32, tag="at", name="at")
                nc.tensor.matmul(pat[:], q_win[:], qt[:, cb, :],
                                 start=True, stop=True)
                nc.scalar.activation(eT[0:C, cb, :], pat[:], AF.Exp,
                                     scale=inv_sqrt_q)
                eng = nc.sync if cb % 2 == 0 else nc.scalar
                eng.dma_start_transpose(e_nat[:, cb, :, :], eT[:, cb, :])
                nc.vector.reduce_sum(dsum[:, cb:cb + 1], eT[0:C, cb, :],
                                     axis=mybir.AxisListType.X)

            emit_eT(0)
            emit_eT(1)
            for cb in range(NCH):
                if cb + 2 < NCH:
                    emit_eT(cb + 2)
                for nb in range(BPC):
                    nc.tensor.matmul(
                        pnum[:], e_nat[:, cb, nb, 0:C],
                        h_nat[:, cb * BPC + nb, :, :].rearrange(
                            "p a b -> p (a b)"),
                        start=(cb == 0 and nb == 0),
                        stop=(cb == NCH - 1 and nb == BPC - 1))

            den = persist.tile([C, 1], F32)
            nc.vector.reduce_sum(den[:], dsum[:],
                                 axis=mybir.AxisListType.X)
            out_sb = persist.tile([C, D + 1], F32)
            nc.scalar.copy(out_sb[:, 0:D], pnum[:])
            nc.vector.tensor_copy(out_sb[:, D:D + 1], den[:])
            nc.sync.dma_start(out_d[:], out_sb[:])

    nc.compile()
    return nc


def _make_in_maps(inputs, n_cores=N_CORES, N_loc=N_LOC):
    x = np.asarray(inputs["x"], dtype=np.float32)
    B = x.shape[0]
    D = int(np.asarray(inputs["W_enc"]).shape[1])
    DB = D // 128

    def bf(a):
        return np.ascontiguousarray(np.asarray(a, np.float32).astype(bfdt))

    def blk(a, last):
        # [K, M] -> [128, K//128, M] (partition-major i-block packing)
        a = np.asarray(a, np.float32)
        return np.ascontiguousarray(
            a.reshape(-1, 128, last).transpose(1, 0, 2).astype(bfdt))

    b_enc = np.asarray(inputs["b_enc"], np.float32)
    b_q1 = np.asarray(inputs["b_q1"], np.float32)
    b_q2 = np.asarray(inputs["b_q2"], np.float32)
    bias = np.zeros((128, DB + 2), np.float32)
    bias[:, 0:DB] = b_enc.reshape(DB, 128).T
    bias[:, DB] = b_q1
    bias[:, DB + 1] = b_q2

    shared = {
        "w_enc": blk(inputs["W_enc"], D),
        "w_i": blk(inputs["W_i"], 2),
        "w_q1": blk(inputs["W_q1"], 128),
        "w_q2": bf(inputs["W_q2"]),
        "bias": bias,
        "identb": np.eye(128, dtype=np.float32).astype(bfdt),
        "identf": np.eye(128, dtype=np.float32),
    }
    xb = x.astype(bfdt)
    NCH = N_loc // 512
    in_maps = []
    for core in range(n_cores):
        bag = core // 2
        half = core % 2
        xh = xb[bag % B, half * N_loc:(half + 1) * N_loc, :]
        # chunk-major: [NCH, 128(p), IB, 512(n)] with 8KB contiguous runs
        xts = np.ascontiguousarray(
            xh.reshape(NCH, 512, -1, 128).transpose(0, 3, 2, 1))
        in_maps.append({"xt": xts, **shared})
    return in_maps


def kernel(**inputs) -> np.ndarray:
    from concourse.bass_utils import run_bass_kernel_spmd

    if "nc" not in _cache:
        _cache["nc"] = _build_kernel()
    nc = _cache["nc"]
    in_maps = _make_in_maps(inputs)
    res = run_bass_kernel_spmd(nc, in_maps, core_ids=list(range(N_CORES)))
    D = 512
    outs = []
    for b in range(B_BAGS):
        pa = res.results[2 * b]["out"].astype(np.float64)
        pb = res.results[2 * b + 1]["out"].astype(np.float64)
        num = pa[:, 0:D] + pb[:, 0:D]
        den = pa[:, D] + pb[:, D]
        outs.append(num / den[:, None])
    return np.stack(outs).astype(np.float32)


# revision 20
# speedup vs baseline: 1.7113x; 1.7113x over previous
"""DSMIL forward pass on 8 Trainium2 NeuronCores (Bass/Tile), bf16 compute.

Sharding: data-parallel over bags, each bag split across a core pair
(core 2b: instances [0:4096) of bag b, core 2b+1: [4096:8192)). Two tiny
pair-local collectives (critical-instance exchange + softmax partial
reduction) keep it a single NEFF launch.

Key implementation choices (vs the fp32r baseline, 292us -> ~152us):
  - x is transposed, cast to bf16 and laid out chunk-major on the HOST:
    the kernel streams xT i-block tiles straight into matmuls (no
    on-chip x transposes, half the HBM traffic, 8KB DMA runs).
  - all matmuls run in bf16 (fp32 PSUM accumulation); the class-score
    path stays fp32 from PSUM onward so the per-class argmax is stable
    (verified against the fp32 reference argmax in numpy simulation).
  - classes are computed in NATURAL [n, c] layout directly
    (lhsT = h^T block, rhs = W_i block), so the per-class max reduction
    runs on all 128 DVE lanes; b_i is dropped (a per-class constant
    shift can never change the per-class argmax, which is the only
    consumer of the scores).
  - the cross-partition max + broadcast is ONE gpsimd
    partition_all_reduce; the onehot compare is ONE DVE op via a
    stride-0 broadcast view.
  - weights/constants load via one consolidated DMA each, spread over
    the gpsimd/scalar queues; x chunks own the sync queue; the first
    x chunk and W_enc are split in half so the first matmul starts as
    early as possible.
  - dummy PE transposes at kernel start pre-warm the HAM clock gate
    while the first DMAs land; a second batch keeps it warm through
    the exchange window.
  - the pair exchanges (m_feats, max) right after the m_feats matmul;
    q_fn runs once on the winner AFTER the select.  The Q = q_fn(h)
    pass is emitted around the exchange so the AllGather is fully
    hidden under real work.
  - the final pair reduction is done on the HOST from per-core
    (num|den) partials, eliminating the second collective and the
    ~20us tail it cost.
"""
import numpy as np
import ml_dtypes
from contextlib import ExitStack

import concourse.bacc as bacc
import concourse.tile as tile
import concourse.mybir as mybir
import concourse.bass_isa as bass_isa

F32 = mybir.dt.float32
BF16 = mybir.dt.bfloat16
AF = mybir.ActivationFunctionType
ALU = mybir.AluOpType
bfdt = ml_dtypes.bfloat16

N_CORES = 8
B_BAGS = 4
N_FULL = 8192
N_LOC = N_FULL // 2

_cache = {}


def _build_kernel(n_cores=N_CORES, N_loc=N_LOC, I=1024, D=512, QD=128,
                  C=2, CHUNK=512, N_WARM0=45, N_WARM=32):
    NB = N_loc // 128          # n-blocks (32)
    NCH = N_loc // CHUNK       # chunks (8)
    BPC = CHUNK // 128         # n-blocks per chunk (4)
    IB = I // 128              # i-blocks (8)
    DB = D // 128              # d-blocks (4)
    assert QD == 128 and C == 2
    inv_sqrt_q = 1.0 / float(np.sqrt(QD))

    nc = bacc.Bacc("TRN2", target_bir_lowering=False, debug=False,
                   num_devices=n_cores)

    xt_d = nc.dram_tensor("xt", [N_loc // CHUNK, 128, I // 128, CHUNK],
                          BF16, kind="ExternalInput")
    w_enc = nc.dram_tensor("w_enc", [128, IB, D], BF16, kind="ExternalInput")
    w_i = nc.dram_tensor("w_i", [128, DB, C], BF16, kind="ExternalInput")
    w_q1 = nc.dram_tensor("w_q1", [128, DB, QD], BF16, kind="ExternalInput")
    w_q2 = nc.dram_tensor("w_q2", [QD, QD], BF16, kind="ExternalInput")
    bias_d = nc.dram_tensor("bias", [128, DB + 2], F32, kind="ExternalInput")
    identb_d = nc.dram_tensor("identb", [128, 128], BF16,
                              kind="ExternalInput")
    identf_d = nc.dram_tensor("identf", [128, 128], F32,
                              kind="ExternalInput")
    out_d = nc.dram_tensor("out", [C, D + 1], F32, kind="ExternalOutput")

    groups = [[i, i + 1] for i in range(0, n_cores, 2)]

    with tile.TileContext(nc) as tc, ExitStack() as ctx:
        persist = ctx.enter_context(tc.tile_pool(name="persist", bufs=1))
        dram = ctx.enter_context(tc.tile_pool(name="dram", bufs=1,
                                              space="DRAM"))

        # ---- scratch consts on the (idle) vector queue ----
        scrap = persist.tile([128, 128], BF16)
        nc.vector.memset(scrap[:], 0.0)
        warm_in = dram.tile([1, 2], F32)
        nc.scalar.dma_start(warm_in[:], identf_d[0:1, 0:2])

        # ---- consolidated weight loads (gpsimd queue) ----
        w_enc_sb = persist.tile([128, IB, D], BF16)
        nc.gpsimd.dma_start(w_enc_sb[:, 0:IB // 2, :], w_enc[:, 0:IB // 2, :])
        nc.gpsimd.dma_start(w_enc_sb[:, IB // 2:, :], w_enc[:, IB // 2:, :])
        w_q1_sb = persist.tile([128, DB, QD], BF16)
        nc.gpsimd.dma_start(w_q1_sb[:], w_q1[:])
        w_i_sb = persist.tile([128, DB, C], BF16)
        nc.gpsimd.dma_start(w_i_sb[:], w_i[:])
        w_q2_sb = persist.tile([QD, QD], BF16)
        nc.gpsimd.dma_start(w_q2_sb[:], w_q2[:])

        # warm both collective channels (fires once weights are queued)
        warm_out = dram.tile([2, 2], F32)
        nc.gpsimd.collective_compute(
            "AllGather", ALU.bypass, replica_groups=groups,
            ins=[warm_in[:].opt()], outs=[warm_out[:].opt()])

        # ---- small consts (scalar queue; DMAs emitted inside chunk 0
        # so the first x half-chunk goes out first) ----
        identb = persist.tile([128, 128], BF16)
        identf = persist.tile([128, 128], F32)
        bias_sb = persist.tile([128, DB + 2], F32)

        # ---- persistent activations ----
        ht_all = persist.tile([128, NCH, DB, CHUNK], BF16)   # h^T
        h_nat = persist.tile([128, NB, D], BF16)             # h natural
        qt = persist.tile([128, NCH, CHUNK], BF16)           # Q^T
        cls_nat = persist.tile([128, NB, C], F32)
        oh = persist.tile([128, NB, C], BF16)
        e_nat = persist.tile([128, NB, C], BF16)

        # ================= phase A: encoder + classes + h transposes ====
        with (
            tc.tile_pool(name="xload", bufs=2) as xload,
            tc.tile_pool(name="hp", bufs=2, space="PSUM") as hp,
            tc.tile_pool(name="tp", bufs=1, space="PSUM") as tp,
            tc.tile_pool(name="cp", bufs=2, space="PSUM") as cp,
        ):
            # pre-warm the PE clock gate while the first DMAs land
            pw = tp.tile([128, D], BF16, tag="t0", name="t")
            for k in range(N_WARM0):
                nc.tensor.transpose(pw[:, 0:128], scrap[:], scrap[:])

            for cb in range(NCH):
                n0 = cb * CHUNK
                xt_c = xload.tile([128, IB, CHUNK], BF16, tag="x", name="x")
                if cb == 0:
                    nc.sync.dma_start(xt_c[:, 0:IB // 2, :],
                                      xt_d[cb][:, 0:IB // 2, :])
                    nc.scalar.dma_start(xt_c[:, IB // 2:, :],
                                        xt_d[cb][:, IB // 2:, :])
                    nc.scalar.dma_start(bias_sb[:], bias_d[:])
                    nc.scalar.dma_start(identb[:], identb_d[:])
                    nc.scalar.dma_start(identf[:], identf_d[:])
                else:
                    nc.sync.dma_start(xt_c[:], xt_d[cb])

                # h^T = relu(W_enc^T @ xT) per d-block; each d-block's
                # h_nat transposes run as soon as ITS relu is done (the
                # next d-block's matmul group covers the relu latency)
                pts = [tp.tile([128, D], BF16, tag=f"t{b}", name="t")
                       for b in range(BPC)]
                for db in range(DB):
                    ph = hp.tile([128, CHUNK], F32, tag="h", name="h")
                    for ib in range(IB):
                        nc.tensor.matmul(
                            ph[:],
                            w_enc_sb[:, ib, db * 128:(db + 1) * 128],
                            xt_c[:, ib, :],
                            start=(ib == 0), stop=(ib == IB - 1))
                    nc.scalar.activation(ht_all[:, cb, db, :], ph[:],
                                         AF.Relu,
                                         bias=bias_sb[:, db:db + 1])
                    if db >= 1:
                        tdb = db - 1
                        for b in range(BPC):
                            nc.tensor.transpose(
                                pts[b][:, tdb * 128:(tdb + 1) * 128],
                                ht_all[:, cb, tdb,
                                       b * 128:(b + 1) * 128],
                                identb[:])
                for b in range(BPC):
                    nc.tensor.transpose(
                        pts[b][:, (DB - 1) * 128:DB * 128],
                        ht_all[:, cb, DB - 1, b * 128:(b + 1) * 128],
                        identb[:])
                for b in range(BPC):
                    nc.vector.tensor_copy(h_nat[:, cb * BPC + b, :],
                                          pts[b][:])

                # classes in natural layout: lhsT = h^T block, rhs = W_i
                pc = cp.tile([128, BPC, C], F32, tag="c", name="c")
                for b in range(BPC):
                    for db in range(DB):
                        nc.tensor.matmul(
                            pc[:, b, :],
                            ht_all[:, cb, db, b * 128:(b + 1) * 128],
                            w_i_sb[:, db, :],
                            start=(db == 0), stop=(db == DB - 1))
                nc.vector.tensor_copy(
                    cls_nat[:, cb * BPC:(cb + 1) * BPC, :], pc[:])

        # ====== exchange prep + Q-pass (hides the AllGather) ===========
        with (
            tc.tile_pool(name="zp", bufs=2, space="PSUM") as zp,
            tc.tile_pool(name="qp", bufs=1, space="PSUM") as qp,
            tc.tile_pool(name="pa", bufs=1, space="PSUM") as pa,
            tc.tile_pool(name="zs", bufs=2) as zs,
        ):
            def q_pass(cb):
                pz = zp.tile([128, CHUNK], F32, tag="z", name="z")
                for db in range(DB):
                    nc.tensor.matmul(pz[:], w_q1_sb[:, db, :],
                                     ht_all[:, cb, db, :],
                                     start=(db == 0), stop=(db == DB - 1))
                zt = zs.tile([128, CHUNK], BF16, tag="zt", name="zt")
                nc.vector.tensor_scalar(zt[:], pz[:],
                                        bias_sb[:, DB:DB + 1], 0.0,
                                        ALU.add, ALU.max)
                pq = qp.tile([128, CHUNK], F32, tag="q", name="q")
                nc.tensor.matmul(pq[:], w_q2_sb[:], zt[:], start=True,
                                 stop=True)
                nc.scalar.activation(qt[:, cb, :], pq[:], AF.Tanh,
                                     bias=bias_sb[:, DB + 1:DB + 2])

            # per-class max across all instances
            rmax = persist.tile([128, C], F32)
            nc.vector.reduce_max(rmax[:],
                                 cls_nat[:].rearrange("p nb c -> p c nb"),
                                 axis=mybir.AxisListType.X)
            q_pass(0)
            # cross-partition max, broadcast to every partition in one op
            mbb = persist.tile([128, 1, C], F32)
            nc.gpsimd.partition_all_reduce(mbb[:, 0, :], rmax[:], 128,
                                           bass_isa.ReduceOp.max)
            nc.vector.tensor_tensor(oh[:], cls_nat[:],
                                    mbb[:].broadcast_to([128, NB, C]),
                                    ALU.is_equal)
            # mval as a [C, 1] column for the exchange payload
            pmt = pa.tile([C, 1], F32, name="pmt")
            nc.tensor.transpose(pmt[:], mbb[0:1, 0, :], identf[0:1, 0:1])
            # m = onehot^T @ h  (critical instance features)
            pmf = pa.tile([C, D], F32, name="pmf")
            for nb in range(NB):
                nc.tensor.matmul(pmf[:], oh[:, nb, :], h_nat[:, nb, :],
                                 start=(nb == 0), stop=(nb == NB - 1))
            # payload = (m | mval); q_fn moves to after the winner select
            pay_sb = persist.tile([C, D + 1], F32)
            nc.scalar.copy(pay_sb[:, 0:D], pmf[:])
            nc.scalar.copy(pay_sb[:, D:D + 1], pmt[:])
            pay1 = dram.tile([C, D + 1], F32)
            nc.scalar.dma_start(pay1[:], pay_sb[:])
            gath1 = dram.tile([2 * C, D + 1], F32)
            nc.gpsimd.collective_compute(
                "AllGather", ALU.bypass, replica_groups=groups,
                ins=[pay1[:].opt()], outs=[gath1[:].opt()])

            for cb in range(1, NCH):
                q_pass(cb)

            # keep the PE clock gate warm while waiting on the collective
            pwm = pa.tile([128, 128], BF16, name="pwm")
            for k in range(N_WARM):
                nc.tensor.transpose(pwm[:], identb[:], identb[:])

        # ================= phase B: winner, q_fn, scores, bag output ===
        with (
            tc.tile_pool(name="pt2", bufs=2, space="PSUM") as pt2,
            tc.tile_pool(name="ep", bufs=2, space="PSUM") as ep,
            tc.tile_pool(name="pb", bufs=1, space="PSUM") as pb,
        ):
            g2 = persist.tile([C, 2, D + 1], F32)
            nc.sync.dma_start(
                g2[:], gath1[:].rearrange("(two p) c -> p two c", p=C))

            # winner-take-all merge of the pair's critical instances
            wA = persist.tile([C, 1], mybir.dt.uint8)
            nc.vector.tensor_tensor(wA[:], g2[:, 0, D:D + 1],
                                    g2[:, 1, D:D + 1], ALU.is_ge)
            m_win = persist.tile([C, D], F32)
            nc.vector.tensor_copy(m_win[:], g2[:, 1, 0:D])
            nc.vector.copy_predicated(m_win[:],
                                      wA[:].broadcast_to([C, D]),
                                      g2[:, 0, 0:D])

            # q_win = q_fn(m_win)
            mT = persist.tile([128, DB, C], BF16)
            for db in range(DB):
                ptm = pt2.tile([128, C], F32, tag="ptm", name="ptm")
                nc.tensor.transpose(ptm[:],
                                    m_win[:, db * 128:(db + 1) * 128],
                                    identf[0:2, 0:2])
                nc.scalar.copy(mT[:, db, :], ptm[:])
            pzm = pb.tile([128, C], F32, name="pzm")
            for db in range(DB):
                nc.tensor.matmul(pzm[:], w_q1_sb[:, db, :], mT[:, db, :],
                                 start=(db == 0), stop=(db == DB - 1))
            zm = persist.tile([128, C], BF16)
            nc.scalar.activation(zm[:], pzm[:], AF.Relu,
                                 bias=bias_sb[:, DB:DB + 1])
            pqc = pb.tile([128, C], F32, name="pqc")
            nc.tensor.matmul(pqc[:], w_q2_sb[:], zm[:], start=True,
                             stop=True)
            q_win = persist.tile([128, C], BF16)
            nc.scalar.activation(q_win[:], pqc[:], AF.Tanh,
                                 bias=bias_sb[:, DB + 1:DB + 2])

            # e = exp(Q @ q_win / sqrt(qd)) in natural layout
            for cb in range(NCH):
                pe_ = ep.tile([128, BPC, C], F32, tag="e", name="e")
                for b in range(BPC):
                    nc.tensor.matmul(
                        pe_[:, b, :],
                        qt[:, cb, b * 128:(b + 1) * 128],
                        q_win[:], start=True, stop=True)
                nc.scalar.activation(
                    e_nat[:, cb * BPC:(cb + 1) * BPC, :], pe_[:],
                    AF.Exp, scale=inv_sqrt_q)

            # numerator: e^T @ h
            pnum = pb.tile([C, D], F32, name="pnum")
            for nb in range(NB):
                nc.tensor.matmul(pnum[:], e_nat[:, nb, :], h_nat[:, nb, :],
                                 start=(nb == 0), stop=(nb == NB - 1))

            # denominator: cross-instance then cross-partition sum
            denp = persist.tile([128, C], F32)
            nc.vector.reduce_sum(denp[:],
                                 e_nat[:].rearrange("p nb c -> p c nb"),
                                 axis=mybir.AxisListType.X)
            pdt = pb.tile([C, 128], F32, name="pdt")
            nc.tensor.transpose(pdt[:], denp[:], identf[:])
            den = persist.tile([C, 1], F32)
            nc.vector.reduce_sum(den[:], pdt[:], axis=mybir.AxisListType.X)

            out_sb = persist.tile([C, D + 1], F32)
            nc.scalar.copy(out_sb[:, 0:D], pnum[:])
            nc.vector.tensor_copy(out_sb[:, D:D + 1], den[:])
            nc.sync.dma_start(out_d[:], out_sb[:])

    nc.compile()
    return nc


def _make_in_maps(inputs, n_cores=N_CORES, N_loc=N_LOC):
    x = np.asarray(inputs["x"], dtype=np.float32)
    B = x.shape[0]
    D = int(np.asarray(inputs["W_enc"]).shape[1])
    DB = D // 128

    def bf(a):
        return np.ascontiguousarray(np.asarray(a, np.float32).astype(bfdt))

    def blk(a, last):
        # [K, M] -> [128, K//128, M] (partition-major i-block packing)
        a = np.asarray(a, np.float32)
        return np.ascontiguousarray(
            a.reshape(-1, 128, last).transpose(1, 0, 2).astype(bfdt))

    b_enc = np.asarray(inputs["b_enc"], np.float32)
    b_q1 = np.asarray(inputs["b_q1"], np.float32)
    b_q2 = np.asarray(inputs["b_q2"], np.float32)
    bias = np.zeros((128, DB + 2), np.float32)
    bias[:, 0:DB] = b_enc.reshape(DB, 128).T
    bias[:, DB] = b_q1
    bias[:, DB + 1] = b_q2

    shared = {
        "w_enc": blk(inputs["W_enc"], D),
        "w_i": blk(inputs["W_i"], 2),
        "w_q1": blk(inputs["W_q1"], 128),
        "w_q2": bf(inputs["W_q2"]),
        "bias": bias,
        "identb": np.eye(128, dtype=np.float32).astype(bfdt),
        "identf": np.eye(128, dtype=np.float32),
    }
    xb = x.astype(bfdt)
    NCH = N_loc // 512
    in_maps = []
    for core in range(n_cores):
        bag = core // 2
        half = core % 2
        xh = xb[bag % B, half * N_loc:(half + 1) * N_loc, :]
        # chunk-major: [NCH, 128(p), IB, 512(n)] with 8KB contiguous runs
        xts = np.ascontiguousarray(
            xh.reshape(NCH, 512, -1, 128).transpose(0, 3, 2, 1))
        in_maps.append({"xt": xts, **shared})
    return in_maps


def kernel(**inputs) -> np.ndarray:
    from concourse.bass_utils import run_bass_kernel_spmd

    if "nc" not in _cache:
        _cache["nc"] = _build_kernel()
    nc = _cache["nc"]
    in_maps = _make_in_maps(inputs)
    res = run_bass_kernel_spmd(nc, in_maps, core_ids=list(range(N_CORES)))
    D = 512
    outs = []
    for b in range(B_BAGS):
        pa = res.results[2 * b]["out"].astype(np.float64)
        pb = res.results[2 * b + 1]["out"].astype(np.float64)
        num = pa[:, 0:D] + pb[:, 0:D]
        den = pa[:, D] + pb[:, D]
        outs.append(num / den[:, None])
    return np.stack(outs).astype(np.float32)

